# revision 1
# baseline (speedup 1.0000x reference)
"""Trainium2 Bass kernel for nn_Plane_refine_block_2 (gnn_message_passing).

Pipeline (8 NeuronCores, points sharded contiguously):
  L1 (device): per-core geometric mask [P, NL] u8 via 3 normalized affine
      margins (plane slab, x-box, y-box; validity folded into scales):
      K=4 matmuls on PE -> DVE abs-max group reduce -> compare.
  host: converts masks to plane-major gather slot lists (index layout only).
  L2 (device): streams f.T shard, 2-layer pointwise MLP on PE/ACT into
      SBUF-resident h.T (+zero sentinel column), GPSIMD ap_gather pulls
      active columns plane-major, DVE reduces each 128-slot tile -> tile
      maxes M [D, TCAP].
  host: final max over per-plane tile ranges and over cores (the
      "all-reduce max over devices" step of the sharding hint).

h >= 0 after ReLU makes 0-filled sentinel slots exact for max-pooling; the
empty-plane -> 0 case of the reference falls out automatically.
"""
import sys

for _p in ("/opt/trn_rl_repo", "/root/.axon_site/_ro/trn_rl_repo"):
    if _p not in sys.path:
        sys.path.insert(0, _p)

import numpy as np

N = 131072
NC = 8
NL = N // NC
P = 64
D = 128
T = 0.05
CH = 512
NCH = NL // CH
TCAP = 416          # gather tiles per core (TCAP*128 slots)
GCH = 2048          # slots per ap_gather instruction
NGI = TCAP * 128 // GCH
SENT = NL           # sentinel column (zeros) in h.T

_STATE = {}


def _build_l1():
    import concourse.bass as bass
    import concourse.mybir as mybir
    import concourse.tile as tile
    from concourse import bacc
    from concourse.bass import ts

    F32 = mybir.dt.float32
    U8 = mybir.dt.uint8
    A = mybir.AluOpType

    nc = bacc.Bacc()
    xyzw = nc.declare_dram_parameter("xyzw", [4, NL], F32, isOutput=False)
    planes_d = nc.declare_dram_parameter("planes", [P, 12], F32, isOutput=False)
    cen_d = nc.declare_dram_parameter("centers", [1, 3], F32, isOutput=False)
    mask_out = nc.declare_dram_parameter("mask_out", [P, NL], U8, isOutput=True)

    with tile.TileContext(nc) as tc:
        with (
            tc.tile_pool(name="prep", bufs=1) as prep,
            tc.tile_pool(name="psum", bufs=2, space="PSUM") as psum,
            tc.tile_pool(name="mwork", bufs=4) as mwork,
            tc.tile_pool(name="mout", bufs=1) as moutp,
        ):
            planes = prep.tile([P, 12], F32)
            cenb = prep.tile([P, 3], F32)
            nc.sync.dma_start(out=planes, in_=planes_d[:])
            pn = planes[:, 0:3]
            pc = planes[:, 3:6]
            pmin = planes[:, 6:9]
            pmax = planes[:, 9:12]
            nc.sync.dma_start(out=cenb, in_=cen_d[:].to_broadcast([P, 3]))

            tmp3 = prep.tile([P, 3], F32)
            nc.vector.tensor_tensor(out=tmp3, in0=pc, in1=cenb, op=A.subtract)
            nc.vector.tensor_tensor(out=tmp3, in0=tmp3, in1=pn, op=A.mult)
            offe = prep.tile([P, 1], F32)
            nc.vector.reduce_sum(out=offe, in_=tmp3, axis=mybir.AxisListType.X)

            Wd = prep.tile([P, 4], F32)
            nc.vector.tensor_scalar_mul(out=Wd[:, 0:3], in0=pn, scalar1=1.0 / T)
            nc.vector.tensor_scalar_mul(out=Wd[:, 3:4], in0=offe, scalar1=-1.0 / T)

            r3 = prep.tile([P, 3], F32)
            c3 = prep.tile([P, 3], F32)
            nc.vector.tensor_tensor(out=r3, in0=pmax, in1=pmin, op=A.subtract)
            nc.vector.tensor_scalar_mul(out=r3, in0=r3, scalar1=0.5)
            nc.vector.tensor_tensor(out=c3, in0=pmax, in1=pmin, op=A.add)
            nc.vector.tensor_scalar_mul(out=c3, in0=c3, scalar1=0.5)
            nc.vector.tensor_tensor(out=c3, in0=c3, in1=cenb, op=A.subtract)
            v3 = prep.tile([P, 3], F32)
            nc.vector.tensor_scalar(out=v3, in0=pmax, scalar1=0.0, scalar2=None,
                                    op0=A.not_equal)
            rinv = prep.tile([P, 3], F32)
            nc.vector.reciprocal(out=rinv, in_=r3)
            s3 = prep.tile([P, 3], F32)
            nc.vector.tensor_tensor(out=s3, in0=v3, in1=rinv, op=A.mult)
            o3 = prep.tile([P, 3], F32)
            nc.vector.tensor_tensor(out=o3, in0=c3, in1=s3, op=A.mult)
            nc.vector.tensor_scalar_mul(out=o3, in0=o3, scalar1=-1.0)

            W3 = prep.tile([P, 3, 4], F32)
            nc.vector.memset(W3, 0.0)
            nc.vector.tensor_copy(out=W3[:, 0, 0:4], in_=Wd)
            nc.vector.tensor_copy(out=W3[:, 1, 0:1], in_=s3[:, 0:1])
            nc.vector.tensor_copy(out=W3[:, 1, 3:4], in_=o3[:, 0:1])
            nc.vector.tensor_copy(out=W3[:, 2, 1:2], in_=s3[:, 1:2])
            nc.vector.tensor_copy(out=W3[:, 2, 3:4], in_=o3[:, 1:2])
            w3d = nc.dram_tensor("w3_scratch", [P, 3, 4], F32)
            nc.sync.dma_start(out=w3d[:], in_=W3)
            lhsT = prep.tile([4, 3, P], F32)
            w3_ap = w3d[:]
            for g in range(3):
                back_ap = bass.AP(tensor=w3_ap.tensor, offset=w3_ap.offset + 4 * g,
                                  ap=[[1, 4], [12, P]])
                nc.sync.dma_start(out=lhsT[:, g, :], in_=back_ap)

            xyz_sb = prep.tile([4, NL], F32)
            nc.sync.dma_start(out=xyz_sb, in_=xyzw[:])

            mask_sb = moutp.tile([P, NL], U8)

            for it in range(NCH // 2):
                ps = psum.tile([128, 3, CH], F32)
                for half in range(2):
                    ci = it * 2 + half
                    for g in range(3):
                        nc.tensor.matmul(
                            out=ps[half * P:(half + 1) * P, g, :],
                            lhsT=lhsT[:, g, :],
                            rhs=xyz_sb[:, ts(ci, CH)],
                            start=True, stop=True,
                        )
                red = mwork.tile([128, CH], F32)
                ps_g_minor = bass.AP(
                    tensor=ps.tensor, offset=ps.offset,
                    ap=[ps.ap[0], [1, CH], [CH, 3]],
                )
                nc.vector.tensor_reduce(out=red, in_=ps_g_minor,
                                        axis=mybir.AxisListType.X, op=A.max,
                                        apply_absolute_value=True)
                for half in range(2):
                    ci = it * 2 + half
                    nc.gpsimd.tensor_scalar(
                        out=mask_sb[:, ts(ci, CH)],
                        in0=red[half * P:(half + 1) * P, :],
                        scalar1=1.0, scalar2=None, op0=A.is_lt)

            nc.sync.dma_start(out=mask_out[:], in_=mask_sb)
    nc.finalize()
    return nc


def _build_l2():
    import concourse.bass as bass
    import concourse.mybir as mybir
    import concourse.tile as tile
    from concourse import bacc
    from concourse.bass import ts

    F32 = mybir.dt.float32
    I16 = mybir.dt.int16
    A = mybir.AluOpType
    AF = mybir.ActivationFunctionType

    nc = bacc.Bacc()
    featT = nc.declare_dram_parameter("featT", [D, NL], F32, isOutput=False)
    w1t_d = nc.declare_dram_parameter("w1t", [D, D], F32, isOutput=False)
    w2t_d = nc.declare_dram_parameter("w2t", [D, D], F32, isOutput=False)
    gb_d = nc.declare_dram_parameter("gb", [D, 4], F32, isOutput=False)
    gidx_d = nc.declare_dram_parameter("gidx", [D, TCAP * 8], I16, isOutput=False)
    out_d = nc.declare_dram_parameter("tile_max", [D, TCAP], F32, isOutput=True)

    with tile.TileContext(nc) as tc:
        with (
            tc.tile_pool(name="const", bufs=1) as cst,
            tc.tile_pool(name="big", bufs=1) as big,
            tc.tile_pool(name="fchunk", bufs=3) as fpool,
            tc.tile_pool(name="h1", bufs=2) as h1pool,
            tc.tile_pool(name="ps", bufs=4, space="PSUM") as psp,
            tc.tile_pool(name="gath", bufs=3) as gpool,
        ):
            w1t = cst.tile([D, D], F32)
            w2t = cst.tile([D, D], F32)
            gb = cst.tile([D, 4], F32)
            gidx = cst.tile([D, TCAP * 8], I16)
            nc.sync.dma_start(out=w1t, in_=w1t_d[:])
            nc.sync.dma_start(out=w2t, in_=w2t_d[:])
            nc.sync.dma_start(out=gb, in_=gb_d[:])
            nc.sync.dma_start(out=gidx, in_=gidx_d[:])

            hT = big.tile([D, NL + 16], F32)
            nc.vector.memset(hT[:, NL:NL + 16], 0.0)

            for ci in range(NCH):
                fch = fpool.tile([D, CH], F32)
                nc.sync.dma_start(out=fch, in_=featT[:, ts(ci, CH)])
                ps1 = psp.tile([D, CH], F32)
                nc.tensor.matmul(out=ps1, lhsT=w1t, rhs=fch, start=True, stop=True)
                h1 = h1pool.tile([D, CH], F32)
                nc.scalar.activation(out=h1, in_=ps1, func=AF.Relu,
                                     bias=gb[:, 1:2], scale=gb[:, 0:1])
                ps2 = psp.tile([D, CH], F32)
                nc.tensor.matmul(out=ps2, lhsT=w2t, rhs=h1, start=True, stop=True)
                nc.scalar.activation(out=hT[:, ts(ci, CH)], in_=ps2, func=AF.Relu,
                                     bias=gb[:, 3:4], scale=gb[:, 2:3])

            M = big.tile([D, TCAP], F32)
            for i in range(NGI):
                gbuf = gpool.tile([D, GCH], F32)
                nc.gpsimd.ap_gather(
                    out_ap=gbuf, in_ap=hT, idxs_ap=gidx[:, ts(i, GCH // 16)],
                    channels=D, num_elems=NL + 16, d=1, num_idxs=GCH)
                g3 = bass.AP(tensor=gbuf.tensor, offset=gbuf.offset,
                             ap=[gbuf.ap[0], [D, GCH // D], [1, D]])
                nc.vector.tensor_reduce(out=M[:, ts(i, GCH // D)], in_=g3,
                                        axis=mybir.AxisListType.X, op=A.max)

            nc.sync.dma_start(out=out_d[:], in_=M)
    nc.finalize()
    return nc


def _get_kernels():
    if "l1" not in _STATE:
        _STATE["l1"] = _build_l1()
        _STATE["l2"] = _build_l2()
    return _STATE["l1"], _STATE["l2"]


def _l1_in_maps(inputs):
    xyz = np.asarray(inputs["xyz"], np.float32)
    planes = np.concatenate([
        np.asarray(inputs["plane_normal"], np.float32),
        np.asarray(inputs["plane_center"], np.float32),
        np.asarray(inputs["plane_min"], np.float32),
        np.asarray(inputs["plane_max"], np.float32)], axis=1)
    cen = np.asarray(inputs["centers"], np.float32).reshape(1, 3)
    maps = []
    for c in range(NC):
        xyzw = np.empty((4, NL), np.float32)
        xyzw[0:3] = xyz[c * NL:(c + 1) * NL].T
        xyzw[3] = 1.0
        maps.append(dict(xyzw=xyzw, planes=planes, centers=cen))
    return maps


def _l2_in_maps(inputs, mask):
    feat = np.asarray(inputs["feature"], np.float32)
    w1t = np.ascontiguousarray(np.asarray(inputs["W1"], np.float32).T)
    w2t = np.ascontiguousarray(np.asarray(inputs["W2"], np.float32).T)
    gb = np.stack([np.asarray(inputs[k], np.float32)
                   for k in ("g1", "b1", "g2", "b2")], axis=1)
    maps, bounds = [], []
    for c in range(NC):
        msk = mask[c * NL:(c + 1) * NL]
        parts = []
        starts = np.zeros(P, np.int32)
        ends = np.zeros(P, np.int32)
        total = 0
        for p in range(P):
            idx = np.nonzero(msk[:, p])[0].astype(np.int16)
            starts[p] = total // 128
            pad = (-len(idx)) % 128
            parts.append(idx)
            if pad:
                parts.append(np.full(pad, SENT, np.int16))
            total += len(idx) + pad
            ends[p] = total // 128
        assert total <= TCAP * 128, f"core {c}: {total} slots exceed capacity"
        if total < TCAP * 128:
            parts.append(np.full(TCAP * 128 - total, SENT, np.int16))
        slots = np.concatenate(parts)
        wrapped = slots.reshape(-1, 16).T
        gidx = np.ascontiguousarray(np.tile(wrapped, (8, 1)))
        featT = np.ascontiguousarray(feat[c * NL:(c + 1) * NL].T)
        maps.append(dict(featT=featT, w1t=w1t, w2t=w2t, gb=gb, gidx=gidx))
        bounds.append((starts, ends))
    return maps, bounds


def run_layers(inputs, trace=False, trace_kwargs=None):
    """Run both launches; returns (pooled [P, D], exec_times_ns list, results)."""
    from concourse.bass_utils import run_bass_kernel_spmd

    l1, l2 = _get_kernels()
    kw = dict(trace=trace)
    if trace_kwargs:
        kw.update(trace_kwargs)
    core_ids = list(range(NC))

    r1 = run_bass_kernel_spmd(l1, _l1_in_maps(inputs), core_ids, **kw)
    mask = np.concatenate(
        [r1.results[c]["mask_out"].T for c in range(NC)], axis=0).astype(bool)

    maps, bounds = _l2_in_maps(inputs, mask)
    r2 = run_bass_kernel_spmd(l2, maps, core_ids, **kw)

    pooled = np.zeros((P, D), np.float32)
    for c in range(NC):
        M = r2.results[c]["tile_max"]
        s, e = bounds[c]
        for p in range(P):
            if e[p] > s[p]:
                np.maximum(pooled[p], M[:, s[p]:e[p]].max(axis=1), out=pooled[p])
    return pooled, [r1.exec_time_ns, r2.exec_time_ns], (r1, r2)


def kernel(**inputs) -> np.ndarray:
    pooled, _, _ = run_layers(inputs, trace=False)
    return pooled
